# revision 9
# baseline (speedup 1.0000x reference)
"""Trainium2 Bass kernel for nn_BusStopPredictor (2-layer GCN + sigmoid head).

kernel(**inputs) takes FULL inputs, shards across 8 NeuronCores internally,
and returns the FULL [200000] output.

Strategy (graph/data parallel, dst-sharded):
  - nodes sharded 8 ways (25000/core, padded to R_BLK=25088 rows/block)
  - per core, per src-bucket b (= shard b), local in-edges grouped into
    degree-class grids (uniform structure across cores, max-padded) so the
    scatter-free segmented reduction is a fixed-stride DVE tensor_reduce
  - gathers via the optimized SWDGE dma_gather (int16 idx, 256B rows,
    per-bucket table slices); per-bucket partial sums combined in HBM via
    dma_scatter_add (CCE add), self-loop terms applied densely on-chip
  - GCN algebra folded so only two aggregations are needed:
      xa    = sum dinv[s]*x[s] (+ dinv*x self)       [N,2]
      xaug  = [dinv^2*xa, dinv];  W1aug=[W1;b1]
      h1'   = relu(xaug@W1aug)   ( = dinv*relu(dinv*(xa@W1)+b1) = dinv*h1 )
      g'    = h1'@W2             ( = dinv*(h1@W2) )  -> AllGather
      out2  = sum g'[s] (+ g'[self]);  h2 = relu(dinv*out2 + b2)
      y     = sigmoid(h2@Wp + bp)
"""

import numpy as np

N = 200000
NCORES = 8
NLOC = N // NCORES          # 25000
P = 128
G_ROWS = 196                # ceil(25000/128)
R_BLK = G_ROWS * P          # 25088 rows per padded block
DG = 64                     # g feature width
TRASH = R_BLK               # trash row index in accum buffers
BUF_ROWS = (G_ROWS + 1) * P  # 25216 (includes trash row area)
CALL_MAX = 1024             # max slots (and rows) per SWDGE call; %128==0 (HW packet cap: 64 desc/lane)
SCRATCH = 16384             # SWDGE descriptor ring carveout (bytes/partition)


# ----------------------------------------------------------------- host prep

def _build_uniform_grids(src, dst):
    """Group each core's in-edges by src bucket and degree class with a
    structure (class/chunk/call layout) identical across cores.

    Returns (calls, percore) where
      calls: list of dicts {bucket, slots, rows, units:[(d, gcnt)],
                            slot_off, row_off} shared by all cores
      percore: list over cores of dict(idx=i16[S_tot], norm=f32[S_tot],
                                       sc=i16[SC_tot])
    """
    # per (core, bucket): srcs of in-edges for each local node
    # counts[c][b] = int array [NLOC] of per-bucket indegree
    edge_core = dst // NLOC
    edge_bucket = src // NLOC
    counts = np.zeros((NCORES, NCORES, NLOC), np.int32)
    for c in range(NCORES):
        em = edge_core == c
        ed = (dst[em] - c * NLOC).astype(np.int64)
        eb = edge_bucket[em]
        for b in range(NCORES):
            bm = eb == b
            counts[c, b] = np.bincount(ed[bm], minlength=NLOC)

    # per-core sorted adjacency for slot filling:
    # adj[c][b] = (starts [NLOC+1], srcs_local sorted by dst)
    adj = [[None] * NCORES for _ in range(NCORES)]
    for c in range(NCORES):
        em = edge_core == c
        ed = (dst[em] - c * NLOC).astype(np.int64)
        es = src[em]
        eb = edge_bucket[em]
        for b in range(NCORES):
            bm = eb == b
            edb = ed[bm]
            esb = (es[bm] - b * NLOC).astype(np.int64)
            o = np.argsort(edb, kind="stable")
            starts = np.zeros(NLOC + 1, np.int64)
            np.cumsum(np.bincount(edb, minlength=NLOC), out=starts[1:])
            adj[c][b] = (starts, esb[o])

    calls = []
    idx_parts = [[] for _ in range(NCORES)]
    norm_parts = [[] for _ in range(NCORES)]
    sc_parts = [[] for _ in range(NCORES)]
    slot_off = 0
    row_off = 0

    for b in range(NCORES):
        # uniform class structure for this bucket: chunks per degree d
        dmax = int(counts[:, b].max())
        nch = {}
        for d in range(1, dmax + 1):
            cnt_d = [(counts[c, b] == d).sum() for c in range(NCORES)]
            m = int(max(cnt_d))
            if m:
                nch[d] = (m + P - 1) // P
        # per-core node lists per class
        nodes_by_class = []
        for c in range(NCORES):
            dloc = counts[c, b]
            nb = {}
            for d in nch:
                nb[d] = np.where(dloc == d)[0]
            nodes_by_class.append(nb)

        # emit calls: pack (d, chunk) units greedily, in increasing d
        pend_units = []   # (d, g) list
        pend_slots = 0

        def flush():
            nonlocal pend_units, pend_slots, slot_off, row_off
            if not pend_units:
                return
            # merge consecutive same-d units into (d, gcnt)
            units = []
            for d, g in pend_units:
                if units and units[-1][0] == d:
                    units[-1][1] += 1
                else:
                    units.append([d, 1])
            rows = sum(g for _, g in units) * P
            calls.append({
                "bucket": b,
                "slots": pend_slots,
                "rows": rows,
                "units": [(d, g) for d, g in units],
                "slot_off": slot_off,
                "row_off": row_off,
            })
            slot_off += pend_slots
            row_off += rows
            pend_units = []
            pend_slots = 0

        for d in sorted(nch):
            for g in range(nch[d]):
                if pend_slots + d * P > CALL_MAX:
                    flush()
                pend_units.append((d, g))
                pend_slots += d * P
        flush()

        # fill per-core slot data for this bucket, in the same (d, g) order
        for c in range(NCORES):
            starts, es_s = adj[c][b]
            for d in sorted(nch):
                nodes = nodes_by_class[c][d]
                padded = nch[d] * P
                nodes_p = np.full(padded, -1, np.int64)
                nodes_p[:len(nodes)] = nodes
                # slots [g, k, p]
                ss = np.zeros((nch[d], d, P), np.int16)
                nv = np.zeros((nch[d], d, P), np.float32)
                scr = np.full((nch[d], P), TRASH, np.int16)
                for g in range(nch[d]):
                    blk = nodes_p[g * P:(g + 1) * P]
                    for p in range(P):
                        ln = blk[p]
                        if ln < 0:
                            continue
                        s0 = starts[ln]
                        ss[g, :, p] = es_s[s0:s0 + d]
                        scr[g, p] = ln
                idx_parts[c].append(ss.reshape(-1))
                sc_parts[c].append(scr.reshape(-1))
                nv_flat = ss.reshape(-1).astype(np.int64) + b * NLOC
                valid = np.repeat(scr.reshape(nch[d], 1, P) != TRASH, d, axis=1)
                norm_parts[c].append((nv_flat, valid.reshape(-1)))

    percore = []
    for c in range(NCORES):
        idx = np.concatenate(idx_parts[c])
        sc = np.concatenate(sc_parts[c])
        percore.append({"idx": idx, "sc": sc, "norm_meta": norm_parts[c]})
    return calls, percore, slot_off, row_off


def _wrap16(vals_i16, pad_unit=128):
    """[S] int16 -> [128, S/16] wrap layout (i%16 partition, i//16 free,
    replicated across the 8 core groups). S must be %128."""
    v = np.asarray(vals_i16, np.int16)
    assert len(v) % pad_unit == 0
    w = v.reshape(len(v) // 16, 16).T
    return np.tile(w, (8, 1))


def _slotmajor(vals, S):
    """[S] -> [128, S/128] layout matching gather output rows (i%128, i//128)."""
    return np.asarray(vals).reshape(S // 128, 128).T.copy()


def _prep(edge_index):
    src = np.asarray(edge_index[0], np.int64)
    dst = np.asarray(edge_index[1], np.int64)
    deg = np.bincount(dst, minlength=N).astype(np.int64) + 1
    dinv = (1.0 / np.sqrt(deg)).astype(np.float32)

    calls, percore, S_tot, R_tot = _build_uniform_grids(src, dst)

    data = []
    for c in range(NCORES):
        pc = percore[c]
        idx_w = _wrap16(pc["idx"])
        sc_w = _wrap16(pc["sc"])
        # per-slot norm (dinv of global src, 0 for pad slots), slot-major
        norm = np.zeros(S_tot, np.float32)
        off = 0
        for nv_flat, valid in pc["norm_meta"]:
            n = len(nv_flat)
            norm[off:off + n] = np.where(valid, dinv[nv_flat], 0.0)
            off += n
        norm_sm = _slotmajor(norm, S_tot)
        data.append({"idx": idx_w, "sc": sc_w, "norm": norm_sm})
    return calls, data, dinv, S_tot, R_tot


# ------------------------------------------------------------- device kernel

def _build_bass(calls, S_tot, R_tot):
    import concourse.bass as bass
    import concourse.mybir as mybir
    import concourse.tile as tile
    from concourse import bacc
    from concourse.masks import make_identity

    F32, I16 = mybir.dt.float32, mybir.dt.int16
    AX = mybir.AxisListType
    OP = mybir.AluOpType
    ACTF = mybir.ActivationFunctionType

    nc = bacc.Bacc(trn_type="TRN2", num_devices=NCORES,
                   dynamic_dma_scratch_size=SCRATCH)

    # inputs
    x_blk = nc.dram_tensor("x_blk", [NCORES, R_BLK, 2], F32, kind="ExternalInput")
    x_self = nc.dram_tensor("x_self", [P, G_ROWS, 2], F32, kind="ExternalInput")
    dinv_pl = nc.dram_tensor("dinv_pl", [P, G_ROWS], F32, kind="ExternalInput")
    w1aug = nc.dram_tensor("w1aug", [3, 128], F32, kind="ExternalInput")
    w2 = nc.dram_tensor("w2", [128, DG], F32, kind="ExternalInput")
    wp_rep = nc.dram_tensor("wp_rep", [P, DG], F32, kind="ExternalInput")
    b2_rep = nc.dram_tensor("b2_rep", [P, DG], F32, kind="ExternalInput")
    bp_rep = nc.dram_tensor("bp_rep", [P, 1], F32, kind="ExternalInput")
    idx_in = nc.dram_tensor("idx", [P, S_tot // 16], I16, kind="ExternalInput")
    norm_in = nc.dram_tensor("norm", [P, S_tot // 128], F32, kind="ExternalInput")
    sc_in = nc.dram_tensor("sc", [P, R_tot // 16], I16, kind="ExternalInput")
    y_out = nc.dram_tensor("y", [R_BLK], F32, kind="ExternalOutput")

    with tile.TileContext(nc) as tc:
        with (
            tc.tile_pool(name="dram", bufs=1, space="DRAM") as dram,
            tc.tile_pool(name="const", bufs=1) as cp,
            tc.tile_pool(name="gath", bufs=3) as gp,
            tc.tile_pool(name="part", bufs=3) as pp,
            tc.tile_pool(name="mm", bufs=3) as mm,
            tc.tile_pool(name="psum", bufs=2, space="PSUM") as ps,
            tc.tile_pool(name="fin", bufs=2) as fp,
        ):
            # DRAM scratch
            x_pad = dram.tile([NCORES * R_BLK, DG], F32)
            xa_buf = dram.tile([BUF_ROWS, DG], F32)
            out2_buf = dram.tile([BUF_ROWS, DG], F32)
            g_mine = dram.tile([R_BLK, DG], F32)
            g_full = dram.tile([NCORES * R_BLK, DG], F32)

            # ---- consts into SBUF ----
            idx_t = cp.tile([P, S_tot // 16], I16)
            nc.sync.dma_start(idx_t[:], idx_in[:])
            sc_t = cp.tile([P, R_tot // 16], I16)
            nc.sync.dma_start(sc_t[:], sc_in[:])
            norm_t = cp.tile([P, S_tot // 128], F32)
            nc.sync.dma_start(norm_t[:], norm_in[:])
            w1_t = cp.tile([P, 128], F32)
            nc.sync.dma_start(w1_t[:3, :], w1aug[:])
            w2_t = cp.tile([P, DG], F32)
            nc.sync.dma_start(w2_t[:], w2[:])
            wp_t = cp.tile([P, DG], F32)
            nc.sync.dma_start(wp_t[:], wp_rep[:])
            b2_t = cp.tile([P, DG], F32)
            nc.sync.dma_start(b2_t[:], b2_rep[:])
            bp_t = cp.tile([P, 1], F32)
            nc.sync.dma_start(bp_t[:], bp_rep[:])
            dv_t = cp.tile([P, G_ROWS], F32)
            nc.sync.dma_start(dv_t[:], dinv_pl[:])
            xs_t = cp.tile([P, G_ROWS, 2], F32)
            nc.sync.dma_start(xs_t[:], x_self[:])
            ident = cp.tile([P, P], F32)
            make_identity(nc, ident[:])
            dv2_t = cp.tile([P, G_ROWS], F32)
            nc.vector.tensor_tensor(out=dv2_t[:], in0=dv_t[:], in1=dv_t[:], op=OP.mult)

            # ---- zero accumulators ----
            zt = cp.tile([P, 16, DG], F32)
            nc.vector.memset(zt[:], 0.0)
            for buf in (xa_buf, out2_buf):
                bv = buf[:].rearrange("(g p) d -> p g d", p=P)
                g = 0
                while g < BUF_ROWS // P:
                    n = min(16, BUF_ROWS // P - g)
                    nc.sync.dma_start(bv[:, g:g + n, :], zt[:, :n, :])
                    g += n

            # ---- build x_pad (cols 0:2 only; cols 2:64 stay garbage) ----
            for b in range(NCORES):
                xt = mm.tile([P, G_ROWS, 2], F32, name="xpb")
                nc.sync.dma_start(xt[:], x_blk[b].rearrange("(g p) d -> p g d", p=P))
                dstv = x_pad[b * R_BLK:(b + 1) * R_BLK, 0:2]
                nc.sync.dma_start(dstv.rearrange("(g p) d -> p g d", p=P), xt[:])

            # ---- helper: one aggregation pass (gather+reduce+scatter) ----
            def agg_pass(table, idx_tile, out_buf, width, norm_tile):
                """width=2: phase1 (use norm, write cols 0:2 of zeroed tile);
                width=DG: phase2 (no norm)."""
                for ci, call in enumerate(calls):
                    b = call["bucket"]
                    S = call["slots"]
                    R = call["rows"]
                    so, ro = call["slot_off"], call["row_off"]
                    gt = gp.tile([P, CALL_MAX // P, DG], F32, name="gt")
                    nc.gpsimd.dma_gather(
                        out_ap=gt[:, :S // P, :],
                        in_ap=table[b * R_BLK:(b + 1) * R_BLK],
                        idxs_ap=idx_tile[:, so // 16:(so + S) // 16],
                        num_idxs=S, num_idxs_reg=S, elem_size=DG,
                    )
                    pt = pp.tile([P, CALL_MAX // P, DG], F32, name="pt")
                    if width == 2:
                        nc.scalar.activation(out=pt[:, :R // P, :],
                                             in_=zt[:, 0:1, :].to_broadcast([P, R // P, DG]),
                                             func=ACTF.Copy)
                        src_m = gp.tile([P, CALL_MAX // P, 2], F32, name="srcm")
                        nrm = norm_tile[:, so // 128:(so + S) // 128]
                        nc.vector.tensor_tensor(
                            out=src_m[:, :S // P, :],
                            in0=gt[:, :S // P, 0:2],
                            in1=nrm.unsqueeze(2).to_broadcast([P, S // P, 2]),
                            op=OP.mult,
                        )
                        red_src = src_m
                        w = 2
                    else:
                        red_src = gt
                        w = DG
                    # per-class segmented reduce
                    sro = 0   # slot-row offset within call
                    rro = 0   # partial-row offset within call
                    for d, gcnt in call["units"]:
                        seg = red_src[:, sro:sro + gcnt * d, :w]
                        seg = seg.rearrange("p (g d) f -> p g f d", d=d)
                        nc.vector.tensor_reduce(
                            out=pt[:, rro:rro + gcnt, :w],
                            in_=seg, axis=AX.X, op=OP.add,
                        )
                        sro += gcnt * d
                        rro += gcnt
                    nc.gpsimd.dma_scatter_add(
                        out_ap=out_buf[:],
                        in_ap=pt[:, :R // P, :],
                        idxs_ap=sc_t[:, ro // 16:(ro + R) // 16],
                        num_idxs=R, num_idxs_reg=R, elem_size=DG,
                    )

            # =================== phase 1 ===================
            agg_pass(x_pad, idx_t, xa_buf, 2, norm_t)

            # readback xa + self + aug
            xa_rb = fp.tile([P, G_ROWS, 2], F32, name="xarb")
            nc.sync.dma_start(
                xa_rb[:], xa_buf[:R_BLK, 0:2].rearrange("(g p) d -> p g d", p=P))
            xaug = cp.tile([P, G_ROWS, 3], F32)
            # xa_total = xa_rb[:, :, 0:2] + dinv*x_self ; xaug01 = xa_total*dinv^2
            tmp2 = fp.tile([P, G_ROWS, 2], F32, name="tmp2")
            nc.vector.tensor_tensor(
                out=tmp2[:], in0=xs_t[:],
                in1=dv_t[:].unsqueeze(2).to_broadcast([P, G_ROWS, 2]), op=OP.mult)
            nc.vector.tensor_tensor(
                out=tmp2[:], in0=tmp2[:], in1=xa_rb[:], op=OP.add)
            nc.vector.tensor_tensor(
                out=xaug[:, :, 0:2], in0=tmp2[:],
                in1=dv2_t[:].unsqueeze(2).to_broadcast([P, G_ROWS, 2]), op=OP.mult)
            nc.vector.tensor_copy(out=xaug[:, :, 2:3], in_=dv_t[:].unsqueeze(2))

            # mm pipeline: per 512-node chunk
            n_chunks = G_ROWS // 4  # 49
            for c in range(n_chunks):
                xT_ps = ps.tile([P, 512], F32, name="xTps", space="PSUM")
                for m in range(4):
                    nc.tensor.transpose(
                        out=xT_ps[0:3, m * 128:(m + 1) * 128],
                        in_=xaug[:, 4 * c + m, :], identity=ident[:])
                xT = mm.tile([P, 512], F32, name="xT")
                nc.scalar.copy(out=xT[0:3, :], in_=xT_ps[0:3, :])
                h_ps = ps.tile([P, 512], F32, name="hps", space="PSUM")
                nc.tensor.matmul(out=h_ps[:], lhsT=w1_t[0:3, :], rhs=xT[0:3, :],
                                 start=True, stop=True)
                h1 = mm.tile([P, 512], F32, name="h1")
                nc.scalar.activation(out=h1[:], in_=h_ps[:], func=ACTF.Relu)
                gsb = mm.tile([P, 4, DG], F32, name="gsb")
                for m in range(4):
                    g_ps = ps.tile([P, DG], F32, name="gps", space="PSUM")
                    nc.tensor.matmul(out=g_ps[:], lhsT=h1[:, m * 128:(m + 1) * 128],
                                     rhs=w2_t[:], start=True, stop=True)
                    nc.vector.tensor_copy(out=gsb[:, m, :], in_=g_ps[:])
                nc.sync.dma_start(
                    g_mine[:].rearrange("(g p) d -> p g d", p=P)[:, 4 * c:4 * c + 4, :],
                    gsb[:])

            # =================== allgather ===================
            nc.gpsimd.collective_compute(
                "AllGather", mybir.AluOpType.bypass,
                replica_groups=[list(range(NCORES))],
                ins=[g_mine[:].opt()], outs=[g_full[:].opt()],
            )

            # =================== phase 2 ===================
            agg_pass(g_full, idx_t, out2_buf, DG, None)

            # final per-node ops, tiled over g-rows (16 at a time)
            GSTEP = 16
            g = 0
            while g < G_ROWS:
                n = min(GSTEP, G_ROWS - g)
                o2 = fp.tile([P, GSTEP, DG], F32, name="o2")
                nc.sync.dma_start(
                    o2[:, :n, :],
                    out2_buf[:R_BLK].rearrange("(g p) d -> p g d", p=P)[:, g:g + n, :])
                gs = fp.tile([P, GSTEP, DG], F32, name="gs")
                nc.sync.dma_start(
                    gs[:, :n, :],
                    g_mine[:].rearrange("(g p) d -> p g d", p=P)[:, g:g + n, :])
                # out2 += g'self ; h2 = relu(out2*dinv + b2)
                nc.vector.tensor_tensor(out=o2[:, :n, :], in0=o2[:, :n, :],
                                        in1=gs[:, :n, :], op=OP.add)
                nc.vector.tensor_tensor(
                    out=o2[:, :n, :], in0=o2[:, :n, :],
                    in1=dv_t[:, g:g + n].unsqueeze(2).to_broadcast([P, n, DG]),
                    op=OP.mult)
                nc.vector.tensor_tensor(
                    out=o2[:, :n, :], in0=o2[:, :n, :],
                    in1=b2_t[:].unsqueeze(1).to_broadcast([P, n, DG]), op=OP.add)
                h2 = fp.tile([P, GSTEP, DG], F32, name="h2")
                nc.scalar.activation(out=h2[:, :n, :], in_=o2[:, :n, :], func=ACTF.Relu)
                # y = sigmoid(sum_f h2*wp + bp)
                nc.vector.tensor_tensor(
                    out=h2[:, :n, :], in0=h2[:, :n, :],
                    in1=wp_t[:].unsqueeze(1).to_broadcast([P, n, DG]), op=OP.mult)
                yt = fp.tile([P, GSTEP], F32, name="yt")
                nc.vector.tensor_reduce(out=yt[:, :n], in_=h2[:, :n, :],
                                        axis=AX.X, op=OP.add)
                ys = fp.tile([P, GSTEP], F32, name="ys")
                nc.scalar.activation(out=ys[:, :n], in_=yt[:, :n],
                                     func=ACTF.Sigmoid, bias=bp_t[:, 0:1])
                nc.sync.dma_start(
                    y_out[:].rearrange("(g p) -> p g", p=P)[:, g:g + n], ys[:, :n])
                g += n

    nc.compile()
    return nc


# ----------------------------------------------------------------- interface

_PROFILE = False      # set by test.py for profiled runs
LAST_EXEC_NS = None


def kernel(x, edge_index, W1, b1, W2, b2, Wp, bp):
    from concourse.bass_utils import run_bass_kernel_spmd

    x = np.asarray(x, np.float32)
    ei = np.asarray(edge_index)
    W1 = np.asarray(W1, np.float32)
    b1 = np.asarray(b1, np.float32)
    W2f = np.asarray(W2, np.float32)
    b2 = np.asarray(b2, np.float32)
    Wp = np.asarray(Wp, np.float32)
    bp = np.asarray(bp, np.float32)

    calls, data, dinv, S_tot, R_tot = _prep(ei)
    nc = _build_bass(calls, S_tot, R_tot)

    # shared inputs
    x_blk = np.zeros((NCORES, R_BLK, 2), np.float32)
    x_blk[:, :NLOC, :] = x.reshape(NCORES, NLOC, 2)
    w1aug = np.concatenate([W1, b1[None, :]], axis=0)
    wp_rep = np.tile(Wp[:, 0][None, :], (P, 1)).astype(np.float32)
    b2_rep = np.tile(b2[None, :], (P, 1)).astype(np.float32)
    bp_rep = np.full((P, 1), bp[0], np.float32)

    in_maps = []
    for c in range(NCORES):
        dv_blk = np.zeros(R_BLK, np.float32)
        dv_blk[:NLOC] = dinv[c * NLOC:(c + 1) * NLOC]
        dinv_pl = dv_blk.reshape(G_ROWS, P).T.copy()
        xs = np.zeros((R_BLK, 2), np.float32)
        xs[:NLOC] = x[c * NLOC:(c + 1) * NLOC]
        x_self = xs.reshape(G_ROWS, P, 2).transpose(1, 0, 2).copy()
        in_maps.append({
            "x_blk": x_blk, "x_self": x_self, "dinv_pl": dinv_pl,
            "w1aug": w1aug, "w2": W2f, "wp_rep": wp_rep,
            "b2_rep": b2_rep, "bp_rep": bp_rep,
            "idx": data[c]["idx"], "norm": data[c]["norm"], "sc": data[c]["sc"],
        })

    global LAST_EXEC_NS
    r = run_bass_kernel_spmd(nc, in_maps, list(range(NCORES)),
                             trace=bool(_PROFILE))
    LAST_EXEC_NS = r.exec_time_ns
    y = np.zeros(N, np.float32)
    for c in range(NCORES):
        y[c * NLOC:(c + 1) * NLOC] = r.results[c]["y"].reshape(R_BLK)[:NLOC]
    return y


# revision 12
# speedup vs baseline: 1.0704x; 1.0704x over previous
"""Trainium2 Bass kernel for nn_BusStopPredictor (2-layer GCN + sigmoid head).

kernel(**inputs) takes FULL inputs, shards across 8 NeuronCores internally,
and returns the FULL [200000] output.

Strategy (graph/data parallel, dst-sharded):
  - nodes sharded 8 ways (25000/core, padded to R_BLK=25088 rows/block)
  - per core, per src-bucket b (= shard b), local in-edges grouped into
    degree-class grids (uniform structure across cores, max-padded) so the
    scatter-free segmented reduction is a fixed-stride DVE tensor_reduce
  - gathers via the optimized SWDGE dma_gather (int16 idx, 256B rows,
    per-bucket table slices); per-bucket partial sums combined in HBM via
    dma_scatter_add (CCE add), self-loop terms applied densely on-chip
  - GCN algebra folded so only two aggregations are needed:
      xa    = sum dinv[s]*x[s] (+ dinv*x self)       [N,2]
      xaug  = [dinv^2*xa, dinv];  W1aug=[W1;b1]
      h1'   = relu(xaug@W1aug)   ( = dinv*relu(dinv*(xa@W1)+b1) = dinv*h1 )
      g'    = h1'@W2             ( = dinv*(h1@W2) )  -> AllGather
      out2  = sum g'[s] (+ g'[self]);  h2 = relu(dinv*out2 + b2)
      y     = sigmoid(h2@Wp + bp)
"""

import numpy as np

N = 200000
NCORES = 8
NLOC = N // NCORES          # 25000
P = 128
G_ROWS = 196                # ceil(25000/128)
R_BLK = G_ROWS * P          # 25088 rows per padded block
DG = 64                     # g feature width
TRASH = R_BLK               # trash row index in accum buffers
BUF_ROWS = (G_ROWS + 1) * P  # 25216 (includes trash row area)
CALL_MAX = 1024             # max slots (and rows) per SWDGE call; %128==0 (HW packet cap: 64 desc/lane)
SCRATCH = 16384             # SWDGE descriptor ring carveout (bytes/partition)


# ----------------------------------------------------------------- host prep

def _build_uniform_grids(src, dst):
    """Group each core's in-edges by src bucket and degree class with a
    structure (class/chunk/call layout) identical across cores.

    Returns (calls, percore) where
      calls: list of dicts {bucket, slots, rows, units:[(d, gcnt)],
                            slot_off, row_off} shared by all cores
      percore: list over cores of dict(idx=i16[S_tot], norm=f32[S_tot],
                                       sc=i16[SC_tot])
    """
    # per (core, bucket): srcs of in-edges for each local node
    # counts[c][b] = int array [NLOC] of per-bucket indegree
    edge_core = dst // NLOC
    edge_bucket = src // NLOC
    counts = np.zeros((NCORES, NCORES, NLOC), np.int32)
    for c in range(NCORES):
        em = edge_core == c
        ed = (dst[em] - c * NLOC).astype(np.int64)
        eb = edge_bucket[em]
        for b in range(NCORES):
            bm = eb == b
            counts[c, b] = np.bincount(ed[bm], minlength=NLOC)

    # per-core sorted adjacency for slot filling:
    # adj[c][b] = (starts [NLOC+1], srcs_local sorted by dst)
    adj = [[None] * NCORES for _ in range(NCORES)]
    for c in range(NCORES):
        em = edge_core == c
        ed = (dst[em] - c * NLOC).astype(np.int64)
        es = src[em]
        eb = edge_bucket[em]
        for b in range(NCORES):
            bm = eb == b
            edb = ed[bm]
            esb = (es[bm] - b * NLOC).astype(np.int64)
            o = np.argsort(edb, kind="stable")
            starts = np.zeros(NLOC + 1, np.int64)
            np.cumsum(np.bincount(edb, minlength=NLOC), out=starts[1:])
            adj[c][b] = (starts, esb[o])

    calls = []
    idx_parts = [[] for _ in range(NCORES)]
    norm_parts = [[] for _ in range(NCORES)]
    sc_parts = [[] for _ in range(NCORES)]
    slot_off = 0
    row_off = 0

    for b in range(NCORES):
        # uniform class structure for this bucket: chunks per degree d
        dmax = int(counts[:, b].max())
        nch = {}
        for d in range(1, dmax + 1):
            cnt_d = [(counts[c, b] == d).sum() for c in range(NCORES)]
            m = int(max(cnt_d))
            if m:
                nch[d] = (m + P - 1) // P
        # per-core node lists per class
        nodes_by_class = []
        for c in range(NCORES):
            dloc = counts[c, b]
            nb = {}
            for d in nch:
                nb[d] = np.where(dloc == d)[0]
            nodes_by_class.append(nb)

        # emit calls: pack (d, chunk) units greedily, in increasing d
        pend_units = []   # (d, g) list
        pend_slots = 0

        def flush():
            nonlocal pend_units, pend_slots, slot_off, row_off
            if not pend_units:
                return
            # merge consecutive same-d units into (d, gcnt)
            units = []
            for d, g in pend_units:
                if units and units[-1][0] == d:
                    units[-1][1] += 1
                else:
                    units.append([d, 1])
            rows = sum(g for _, g in units) * P
            calls.append({
                "bucket": b,
                "slots": pend_slots,
                "rows": rows,
                "units": [(d, g) for d, g in units],
                "slot_off": slot_off,
                "row_off": row_off,
            })
            slot_off += pend_slots
            row_off += rows
            pend_units = []
            pend_slots = 0

        for d in sorted(nch):
            for g in range(nch[d]):
                if pend_slots + d * P > CALL_MAX:
                    flush()
                pend_units.append((d, g))
                pend_slots += d * P
        flush()

        # fill per-core slot data for this bucket, in the same (d, g) order
        for c in range(NCORES):
            starts, es_s = adj[c][b]
            for d in sorted(nch):
                nodes = nodes_by_class[c][d]
                padded = nch[d] * P
                nodes_p = np.full(padded, -1, np.int64)
                nodes_p[:len(nodes)] = nodes
                # slots [g, k, p]
                ss = np.zeros((nch[d], d, P), np.int16)
                nv = np.zeros((nch[d], d, P), np.float32)
                scr = np.full((nch[d], P), TRASH, np.int16)
                for g in range(nch[d]):
                    blk = nodes_p[g * P:(g + 1) * P]
                    for p in range(P):
                        ln = blk[p]
                        if ln < 0:
                            continue
                        s0 = starts[ln]
                        ss[g, :, p] = es_s[s0:s0 + d]
                        scr[g, p] = ln
                idx_parts[c].append(ss.reshape(-1))
                sc_parts[c].append(scr.reshape(-1))
                nv_flat = ss.reshape(-1).astype(np.int64) + b * NLOC
                valid = np.repeat(scr.reshape(nch[d], 1, P) != TRASH, d, axis=1)
                norm_parts[c].append((nv_flat, valid.reshape(-1)))

    percore = []
    for c in range(NCORES):
        idx = np.concatenate(idx_parts[c])
        sc = np.concatenate(sc_parts[c])
        percore.append({"idx": idx, "sc": sc, "norm_meta": norm_parts[c]})
    return calls, percore, slot_off, row_off


def _wrap16(vals_i16, pad_unit=128):
    """[S] int16 -> [128, S/16] wrap layout (i%16 partition, i//16 free,
    replicated across the 8 core groups). S must be %128."""
    v = np.asarray(vals_i16, np.int16)
    assert len(v) % pad_unit == 0
    w = v.reshape(len(v) // 16, 16).T
    return np.tile(w, (8, 1))


def _slotmajor(vals, S):
    """[S] -> [128, S/128] layout matching gather output rows (i%128, i//128)."""
    return np.asarray(vals).reshape(S // 128, 128).T.copy()


def _prep(edge_index):
    src = np.asarray(edge_index[0], np.int64)
    dst = np.asarray(edge_index[1], np.int64)
    deg = np.bincount(dst, minlength=N).astype(np.int64) + 1
    dinv = (1.0 / np.sqrt(deg)).astype(np.float32)

    calls, percore, S_tot, R_tot = _build_uniform_grids(src, dst)

    data = []
    for c in range(NCORES):
        pc = percore[c]
        idx_w = _wrap16(pc["idx"])
        sc_w = _wrap16(pc["sc"])
        # per-slot norm (dinv of global src, 0 for pad slots), slot-major
        norm = np.zeros(S_tot, np.float32)
        off = 0
        for nv_flat, valid in pc["norm_meta"]:
            n = len(nv_flat)
            norm[off:off + n] = np.where(valid, dinv[nv_flat], 0.0)
            off += n
        norm_sm = _slotmajor(norm, S_tot)
        data.append({"idx": idx_w, "sc": sc_w, "norm": norm_sm})
    return calls, data, dinv, S_tot, R_tot


# ------------------------------------------------------------- device kernel

def _build_bass(calls, S_tot, R_tot):
    import concourse.bass as bass
    import concourse.mybir as mybir
    import concourse.tile as tile
    from concourse import bacc
    from concourse.masks import make_identity

    F32, I16 = mybir.dt.float32, mybir.dt.int16
    AX = mybir.AxisListType
    OP = mybir.AluOpType
    ACTF = mybir.ActivationFunctionType

    nc = bacc.Bacc(trn_type="TRN2", num_devices=NCORES,
                   dynamic_dma_scratch_size=SCRATCH)

    # inputs
    x_blk = nc.dram_tensor("x_blk", [NCORES, P, G_ROWS, 2], F32, kind="ExternalInput")
    x_self = nc.dram_tensor("x_self", [P, G_ROWS, 2], F32, kind="ExternalInput")
    dinv_pl = nc.dram_tensor("dinv_pl", [P, G_ROWS], F32, kind="ExternalInput")
    w1aug = nc.dram_tensor("w1aug", [3, 128], F32, kind="ExternalInput")
    w2 = nc.dram_tensor("w2", [128, DG], F32, kind="ExternalInput")
    wp_rep = nc.dram_tensor("wp_rep", [P, DG], F32, kind="ExternalInput")
    b2_rep = nc.dram_tensor("b2_rep", [P, DG], F32, kind="ExternalInput")
    bp_rep = nc.dram_tensor("bp_rep", [P, 1], F32, kind="ExternalInput")
    idx_in = nc.dram_tensor("idx", [P, S_tot // 16], I16, kind="ExternalInput")
    norm_in = nc.dram_tensor("norm", [P, S_tot // 128], F32, kind="ExternalInput")
    sc_in = nc.dram_tensor("sc", [P, R_tot // 16], I16, kind="ExternalInput")
    y_out = nc.dram_tensor("y", [R_BLK], F32, kind="ExternalOutput")

    with tile.TileContext(nc) as tc:
        with (
            tc.tile_pool(name="dram", bufs=1, space="DRAM") as dram,
            tc.tile_pool(name="const", bufs=1) as cp,
            tc.tile_pool(name="gath", bufs=3) as gp,
            tc.tile_pool(name="part", bufs=3) as pp,
            tc.tile_pool(name="mm", bufs=3) as mm,
            tc.tile_pool(name="psum", bufs=2, space="PSUM") as ps,
            tc.tile_pool(name="fin", bufs=2) as fp,
        ):
            # DRAM scratch
            x_pad = dram.tile([NCORES * R_BLK, DG], F32)
            xa_bufs = [dram.tile([BUF_ROWS, DG], F32, name=f"xa_buf{i}") for i in range(2)]
            out2_bufs = [dram.tile([BUF_ROWS, DG], F32, name=f"out2_buf{i}") for i in range(2)]
            g_mine = dram.tile([R_BLK, DG], F32)
            g_full = dram.tile([NCORES * R_BLK, DG], F32)

            # ---- consts into SBUF ----
            idx_t = cp.tile([P, S_tot // 16], I16)
            nc.sync.dma_start(idx_t[:], idx_in[:])
            sc_t = cp.tile([P, R_tot // 16], I16)
            nc.sync.dma_start(sc_t[:], sc_in[:])
            norm_t = cp.tile([P, S_tot // 128], F32)
            nc.sync.dma_start(norm_t[:], norm_in[:])
            w1_t = cp.tile([P, 128], F32)
            nc.sync.dma_start(w1_t[:3, :], w1aug[:])
            w2_t = cp.tile([P, DG], F32)
            nc.sync.dma_start(w2_t[:], w2[:])
            wp_t = cp.tile([P, DG], F32)
            nc.sync.dma_start(wp_t[:], wp_rep[:])
            b2_t = cp.tile([P, DG], F32)
            nc.sync.dma_start(b2_t[:], b2_rep[:])
            bp_t = cp.tile([P, 1], F32)
            nc.sync.dma_start(bp_t[:], bp_rep[:])
            dv_t = cp.tile([P, G_ROWS], F32)
            nc.sync.dma_start(dv_t[:], dinv_pl[:])
            xs_t = cp.tile([P, G_ROWS, 2], F32)
            nc.sync.dma_start(xs_t[:], x_self[:])
            ident = cp.tile([P, P], F32)
            make_identity(nc, ident[:])
            dv2_t = cp.tile([P, G_ROWS], F32)
            nc.vector.tensor_tensor(out=dv2_t[:], in0=dv_t[:], in1=dv_t[:], op=OP.mult)

            # ---- zero accumulators ----
            zt = cp.tile([P, 16, DG], F32)
            nc.vector.memset(zt[:], 0.0)
            for buf in (*xa_bufs, *out2_bufs):
                bv = buf[:].rearrange("(g p) d -> p g d", p=P)
                g = 0
                while g < BUF_ROWS // P:
                    n = min(16, BUF_ROWS // P - g)
                    nc.sync.dma_start(bv[:, g:g + n, :], zt[:, :n, :])
                    g += n

            # ---- build x_pad: full 64-wide rows (zeros in cols 2:64),
            # contiguous writes; x_blk arrives pre-swizzled [P, G_ROWS, 2]
            GSUB = 49
            xz = cp.tile([P, GSUB, DG], F32)
            nc.vector.memset(xz[:], 0.0)
            for b in range(NCORES):
                for j in range(G_ROWS // GSUB):
                    xt = mm.tile([P, GSUB, 2], F32, name="xpb")
                    nc.sync.dma_start(xt[:], x_blk[b, :, j * GSUB:(j + 1) * GSUB, :])
                    nc.vector.tensor_copy(out=xz[:, :, 0:2], in_=xt[:])
                    rows = x_pad[b * R_BLK + j * GSUB * P:
                                 b * R_BLK + (j + 1) * GSUB * P, :]
                    nc.sync.dma_start(rows.rearrange("(g p) d -> p g d", p=P), xz[:])

            # ---- helper: one aggregation pass (gather+reduce+scatter) ----
            def agg_pass(table, idx_tile, out_bufs, width, norm_tile):
                """width=2: phase1 (use norm, write cols 0:2 of zeroed tile);
                width=DG: phase2 (no norm)."""
                for ci, call in enumerate(calls):
                    b = call["bucket"]
                    S = call["slots"]
                    R = call["rows"]
                    so, ro = call["slot_off"], call["row_off"]
                    gt = gp.tile([P, CALL_MAX // P, DG], F32, name="gt")
                    nc.gpsimd.dma_gather(
                        out_ap=gt[:, :S // P, :],
                        in_ap=table[b * R_BLK:(b + 1) * R_BLK],
                        idxs_ap=idx_tile[:, so // 16:(so + S) // 16],
                        num_idxs=S, num_idxs_reg=S, elem_size=DG,
                    )
                    pt = pp.tile([P, CALL_MAX // P, DG], F32, name="pt")
                    if width == 2:
                        nc.scalar.activation(out=pt[:, :R // P, :],
                                             in_=zt[:, 0:1, :].to_broadcast([P, R // P, DG]),
                                             func=ACTF.Copy)
                        src_m = gp.tile([P, CALL_MAX // P, 2], F32, name="srcm")
                        nrm = norm_tile[:, so // 128:(so + S) // 128]
                        nc.vector.tensor_tensor(
                            out=src_m[:, :S // P, :],
                            in0=gt[:, :S // P, 0:2],
                            in1=nrm.unsqueeze(2).to_broadcast([P, S // P, 2]),
                            op=OP.mult,
                        )
                        red_src = src_m
                        w = 2
                    else:
                        red_src = gt
                        w = DG
                    # per-class segmented reduce
                    sro = 0   # slot-row offset within call
                    rro = 0   # partial-row offset within call
                    for d, gcnt in call["units"]:
                        seg = red_src[:, sro:sro + gcnt * d, :w]
                        seg = seg.rearrange("p (g d) f -> p g f d", d=d)
                        nc.vector.tensor_reduce(
                            out=pt[:, rro:rro + gcnt, :w],
                            in_=seg, axis=AX.X, op=OP.add,
                        )
                        sro += gcnt * d
                        rro += gcnt
                    nc.gpsimd.dma_scatter_add(
                        out_ap=out_bufs[ci % 2][:],
                        in_ap=pt[:, :R // P, :],
                        idxs_ap=sc_t[:, ro // 16:(ro + R) // 16],
                        num_idxs=R, num_idxs_reg=R, elem_size=DG,
                    )

            # =================== phase 1 ===================
            agg_pass(x_pad, idx_t, xa_bufs, 2, norm_t)

            # readback xa + self + aug
            xa_rb = fp.tile([P, G_ROWS, 2], F32, name="xarb")
            nc.sync.dma_start(
                xa_rb[:], xa_bufs[0][:R_BLK, 0:2].rearrange("(g p) d -> p g d", p=P))
            xa_rb1 = fp.tile([P, G_ROWS, 2], F32, name="xarb1")
            nc.sync.dma_start(
                xa_rb1[:], xa_bufs[1][:R_BLK, 0:2].rearrange("(g p) d -> p g d", p=P))
            nc.vector.tensor_tensor(out=xa_rb[:], in0=xa_rb[:], in1=xa_rb1[:],
                                    op=OP.add)
            xaug = cp.tile([P, G_ROWS, 3], F32)
            # xa_total = xa_rb[:, :, 0:2] + dinv*x_self ; xaug01 = xa_total*dinv^2
            tmp2 = fp.tile([P, G_ROWS, 2], F32, name="tmp2")
            nc.vector.tensor_tensor(
                out=tmp2[:], in0=xs_t[:],
                in1=dv_t[:].unsqueeze(2).to_broadcast([P, G_ROWS, 2]), op=OP.mult)
            nc.vector.tensor_tensor(
                out=tmp2[:], in0=tmp2[:], in1=xa_rb[:], op=OP.add)
            nc.vector.tensor_tensor(
                out=xaug[:, :, 0:2], in0=tmp2[:],
                in1=dv2_t[:].unsqueeze(2).to_broadcast([P, G_ROWS, 2]), op=OP.mult)
            nc.vector.tensor_copy(out=xaug[:, :, 2:3], in_=dv_t[:].unsqueeze(2))

            # mm pipeline: per 512-node chunk
            n_chunks = G_ROWS // 4  # 49
            for c in range(n_chunks):
                xT_ps = ps.tile([P, 512], F32, name="xTps", space="PSUM")
                for m in range(4):
                    nc.tensor.transpose(
                        out=xT_ps[0:3, m * 128:(m + 1) * 128],
                        in_=xaug[:, 4 * c + m, :], identity=ident[:])
                xT = mm.tile([P, 512], F32, name="xT")
                nc.scalar.copy(out=xT[0:3, :], in_=xT_ps[0:3, :])
                h_ps = ps.tile([P, 512], F32, name="hps", space="PSUM")
                nc.tensor.matmul(out=h_ps[:], lhsT=w1_t[0:3, :], rhs=xT[0:3, :],
                                 start=True, stop=True)
                h1 = mm.tile([P, 512], F32, name="h1")
                nc.scalar.activation(out=h1[:], in_=h_ps[:], func=ACTF.Relu)
                gsb = mm.tile([P, 4, DG], F32, name="gsb")
                for m in range(4):
                    g_ps = ps.tile([P, DG], F32, name="gps", space="PSUM")
                    nc.tensor.matmul(out=g_ps[:], lhsT=h1[:, m * 128:(m + 1) * 128],
                                     rhs=w2_t[:], start=True, stop=True)
                    nc.vector.tensor_copy(out=gsb[:, m, :], in_=g_ps[:])
                nc.sync.dma_start(
                    g_mine[:].rearrange("(g p) d -> p g d", p=P)[:, 4 * c:4 * c + 4, :],
                    gsb[:])

            # =================== allgather ===================
            nc.gpsimd.collective_compute(
                "AllGather", mybir.AluOpType.bypass,
                replica_groups=[list(range(NCORES))],
                ins=[g_mine[:].opt()], outs=[g_full[:].opt()],
            )

            # =================== phase 2 ===================
            agg_pass(g_full, idx_t, out2_bufs, DG, None)

            # final per-node ops, tiled over g-rows (16 at a time)
            GSTEP = 16
            g = 0
            while g < G_ROWS:
                n = min(GSTEP, G_ROWS - g)
                o2 = fp.tile([P, GSTEP, DG], F32, name="o2")
                nc.sync.dma_start(
                    o2[:, :n, :],
                    out2_bufs[0][:R_BLK].rearrange("(g p) d -> p g d", p=P)[:, g:g + n, :])
                o2b = fp.tile([P, GSTEP, DG], F32, name="o2b")
                nc.sync.dma_start(
                    o2b[:, :n, :],
                    out2_bufs[1][:R_BLK].rearrange("(g p) d -> p g d", p=P)[:, g:g + n, :])
                nc.vector.tensor_tensor(out=o2[:, :n, :], in0=o2[:, :n, :],
                                        in1=o2b[:, :n, :], op=OP.add)
                gs = fp.tile([P, GSTEP, DG], F32, name="gs")
                nc.sync.dma_start(
                    gs[:, :n, :],
                    g_mine[:].rearrange("(g p) d -> p g d", p=P)[:, g:g + n, :])
                # out2 += g'self ; h2 = relu(out2*dinv + b2)
                nc.vector.tensor_tensor(out=o2[:, :n, :], in0=o2[:, :n, :],
                                        in1=gs[:, :n, :], op=OP.add)
                nc.vector.tensor_tensor(
                    out=o2[:, :n, :], in0=o2[:, :n, :],
                    in1=dv_t[:, g:g + n].unsqueeze(2).to_broadcast([P, n, DG]),
                    op=OP.mult)
                nc.vector.tensor_tensor(
                    out=o2[:, :n, :], in0=o2[:, :n, :],
                    in1=b2_t[:].unsqueeze(1).to_broadcast([P, n, DG]), op=OP.add)
                h2 = fp.tile([P, GSTEP, DG], F32, name="h2")
                nc.scalar.activation(out=h2[:, :n, :], in_=o2[:, :n, :], func=ACTF.Relu)
                # y = sigmoid(sum_f h2*wp + bp)
                nc.vector.tensor_tensor(
                    out=h2[:, :n, :], in0=h2[:, :n, :],
                    in1=wp_t[:].unsqueeze(1).to_broadcast([P, n, DG]), op=OP.mult)
                yt = fp.tile([P, GSTEP], F32, name="yt")
                nc.vector.tensor_reduce(out=yt[:, :n], in_=h2[:, :n, :],
                                        axis=AX.X, op=OP.add)
                ys = fp.tile([P, GSTEP], F32, name="ys")
                nc.scalar.activation(out=ys[:, :n], in_=yt[:, :n],
                                     func=ACTF.Sigmoid, bias=bp_t[:, 0:1])
                nc.sync.dma_start(
                    y_out[:].rearrange("(g p) -> p g", p=P)[:, g:g + n], ys[:, :n])
                g += n

    nc.compile()
    return nc


# ----------------------------------------------------------------- interface

_PROFILE = False      # set by test.py for profiled runs
LAST_EXEC_NS = None


def kernel(x, edge_index, W1, b1, W2, b2, Wp, bp):
    from concourse.bass_utils import run_bass_kernel_spmd

    x = np.asarray(x, np.float32)
    ei = np.asarray(edge_index)
    W1 = np.asarray(W1, np.float32)
    b1 = np.asarray(b1, np.float32)
    W2f = np.asarray(W2, np.float32)
    b2 = np.asarray(b2, np.float32)
    Wp = np.asarray(Wp, np.float32)
    bp = np.asarray(bp, np.float32)

    calls, data, dinv, S_tot, R_tot = _prep(ei)
    nc = _build_bass(calls, S_tot, R_tot)

    # shared inputs
    x_blk = np.zeros((NCORES, R_BLK, 2), np.float32)
    x_blk[:, :NLOC, :] = x.reshape(NCORES, NLOC, 2)
    # pre-swizzle to (p, g) layout so device reads are dense
    x_blk = x_blk.reshape(NCORES, G_ROWS, P, 2).transpose(0, 2, 1, 3).copy()
    w1aug = np.concatenate([W1, b1[None, :]], axis=0)
    wp_rep = np.tile(Wp[:, 0][None, :], (P, 1)).astype(np.float32)
    b2_rep = np.tile(b2[None, :], (P, 1)).astype(np.float32)
    bp_rep = np.full((P, 1), bp[0], np.float32)

    in_maps = []
    for c in range(NCORES):
        dv_blk = np.zeros(R_BLK, np.float32)
        dv_blk[:NLOC] = dinv[c * NLOC:(c + 1) * NLOC]
        dinv_pl = dv_blk.reshape(G_ROWS, P).T.copy()
        xs = np.zeros((R_BLK, 2), np.float32)
        xs[:NLOC] = x[c * NLOC:(c + 1) * NLOC]
        x_self = xs.reshape(G_ROWS, P, 2).transpose(1, 0, 2).copy()
        in_maps.append({
            "x_blk": x_blk, "x_self": x_self, "dinv_pl": dinv_pl,
            "w1aug": w1aug, "w2": W2f, "wp_rep": wp_rep,
            "b2_rep": b2_rep, "bp_rep": bp_rep,
            "idx": data[c]["idx"], "norm": data[c]["norm"], "sc": data[c]["sc"],
        })

    global LAST_EXEC_NS
    r = run_bass_kernel_spmd(nc, in_maps, list(range(NCORES)),
                             trace=bool(_PROFILE))
    LAST_EXEC_NS = r.exec_time_ns
    y = np.zeros(N, np.float32)
    for c in range(NCORES):
        y[c * NLOC:(c + 1) * NLOC] = r.results[c]["y"].reshape(R_BLK)[:NLOC]
    return y
